# revision 63
# baseline (speedup 1.0000x reference)
"""Trainium2 Bass kernel for an 8-layer dense MLP (784->512x6->10) + softmax.

Strategy (hardcoded for batch=65536, 8 NeuronCores, pure data parallel):
  - Each core handles 8192 rows of the batch; weights replicated.
  - All matmuls run in fp8-e4m3 with MatmulPerfMode.DoubleRow (256-feature
    contraction per instruction; one 512-px matmul issues every ~216ns =
    the fp8 peak).  PE is the bottleneck: everything else is scheduled to
    keep its 216ns cadence unbroken.
  - Layer 1 contracts 768 of the 784 input features with 3 DoubleRow chunks
    and handles the 16 leftover features as K=16 normal-mode matmuls packed
    two-concurrent onto 32-row array quadrants via tile_position - saving
    most of the pad-to-1024 waste.
  - Quantization scales: per-layer power-of-2 activation scales s_l picked by
    a 2048-row fp32 calibration forward on host; weight scale for layer l is
    exactly s_l/s_{l-1}, so PSUM already carries s_l * preactivation and the
    PSUM->fp8 step is a single fused  q8(max(psum + s_l*b_l, 0))  on either
    ACT (activation, bias=) or DVE (tensor_scalar add,max).
  - FOUR pairs (of two BT=512 batch tiles) are in flight, interleaved at
    layer granularity: every relu/AND -> next-layer dependency gets three
    sibling pair-layer windows (~10us) of slack.  Drop layers alternate the
    relu split (3 ACT/1 DVE vs 2/2+1) per pair parity so neither post-op
    engine runs saturated for four consecutive windows.
  - Dropout masks (jax threefry, key 42) are bit-exactly precomputed on host
    and shipped as u32 {0x00,0xFF}-byte words; applied as ONE whole-layer
    bitwise-AND on DVE whose emission is deferred past the next pair's
    relus (in-order DVE FIFO: an early AND delays the critical relus).
    1/(1-p) is folded into the next layer's weights on host.
  - Softmax: exp on ACT -> bf16 (scale=g8 dequant, bias=b8), replicated
    class-sum via a [10,10] all-ones matmul on PE, reciprocal_approx_fast on
    DVE + multiply on Pool.  Tails are deferred and flushed in the next
    group's roomy L1 windows; the final group drains them staggered between
    the last pairs' L7/L8 so the kernel tail stays short.
  - All PSUM flows through one 4-slot ring of [128,2,BT] two-bank pair
    tiles (8 banks total).  Startup: x + hidden weights + masks stream on
    the Pool software-DGE queue (~250 GB/s measured; HW-DGE queues only
    ~78 GB/s) in first-use order; only the small w1/bias tiles ride the
    HW-DGE queues, and nothing with a semaphore wait is ever placed on a
    compute engine's queue (a gated DMA trigger head-of-line-blocks it).
"""

import numpy as np
import ml_dtypes

E4 = ml_dtypes.float8_e4m3

BATCH = 65536
D_IN = 784
K1_MAIN = 768              # input features contracted via 3 DoubleRow pairs
K1P = K1_MAIN // 256       # = 3 DR chunk-pairs for layer 1
K1_TAIL = D_IN - K1_MAIN   # = 16 leftover features, row-quadrant packed
H = 512
KO = H // 128              # 4 feature chunks for hidden layers
C = 10
N_CORES = 8
B_CORE = BATCH // N_CORES  # 8192
BT = 512                   # batch tile (matmul moving free dim)
PW = 2 * BT                # pair width

DROP_LAYERS = (2, 4, 6)    # dropout applied to these layers' outputs
KEEP = {2: 0.8, 4: 0.7, 6: 0.5}


def build_bass(b_core: int, g8: float):
    """Build the Bass module for one core processing b_core batch rows."""
    import concourse.mybir as mybir
    import concourse.tile as tile
    from concourse import bacc

    f32 = mybir.dt.float32
    f8 = mybir.dt.float8e4
    bf16 = mybir.dt.bfloat16
    AF = mybir.ActivationFunctionType
    ALU = mybir.AluOpType
    PM = mybir.MatmulPerfMode

    npair = b_core // PW

    nc = bacc.Bacc("TRN2", target_bir_lowering=False, debug=False)

    # Weights are pre-packed on host so every DoubleRow lhsT block
    # [128, 2, ncol] is contiguous in SBUF (s3_lw_dual_fp8_restrictions):
    # layer l image is [128, pairs, KO, 2, 128] flattened to 2D.
    # x and masks are pre-packed into the SBUF pair layout
    # [p, pair, ko_pair, tile, slot, BT] flattened to 2D so each pair loads
    # with one fully-contiguous DMA.
    xT = nc.dram_tensor("xT", [128, npair * K1P * 2 * 2 * BT], f8, kind="ExternalInput")
    xtl_h = nc.dram_tensor("xtl", [128, npair * 2 * BT], f8, kind="ExternalInput")
    w_h = {1: nc.dram_tensor("w1", [128, K1P * KO * 256], f8, kind="ExternalInput")}
    w1t_h = nc.dram_tensor("w1t", [128, 128], f8, kind="ExternalInput")
    for l in range(2, 8):
        w_h[l] = nc.dram_tensor(f"w{l}", [128, (KO // 2) * KO * 256], f8, kind="ExternalInput")
    w8_h = nc.dram_tensor("w8", [128, (KO // 2) * 2 * 16], f8, kind="ExternalInput")
    bias17_h = nc.dram_tensor("bias17", [128, 28], f32, kind="ExternalInput")
    b8c_h = nc.dram_tensor("b8c", [128, 1], f32, kind="ExternalInput")
    u32 = mybir.dt.uint32
    # masks ship as uint32 {0x00.., 0xFF..} bytes covering four fp8 lanes
    # each: the dropout multiply is a bitwise AND on u32 views (DVE only —
    # Pool has no bitwise ops and DVE bitwise requires 32-bit).
    m_h = {
        l: nc.dram_tensor(f"m{l}", [128, npair * KO * BT // 2], u32, kind="ExternalInput")
        for l in DROP_LAYERS
    }
    y_h = nc.dram_tensor("yT", [C, b_core], f32, kind="ExternalOutput")

    # Per-layer relu/quantize engine map (A=ACT, D=DVE), one char per output
    # block n=0..3.  Drop layers alternate the 3:1 split per pair parity so
    # neither engine runs >99% for four consecutive windows (the mask-AND
    # rides DVE); other layers split evenly.  Block 0 is always ACT-first:
    # the next pair-layer's first chain WAR-waits on it via the PSUM ring.
    RELU_ENG = {
        1: "ADAD", 2: ("AADA", "ADDA"), 3: "ADAD", 4: ("AADA", "ADDA"),
        5: "ADAD", 6: ("AADA", "ADDA"), 7: "ADAD",
    }

    with tile.TileContext(nc) as tc:
        with (
            tc.tile_pool(name="wpool", bufs=1) as wpool,
            tc.tile_pool(name="xpool", bufs=5) as xpool,
            tc.tile_pool(name="hpool", bufs=8) as hpool,
            tc.tile_pool(name="mpool", bufs=5) as mpool,
            tc.tile_pool(name="spool", bufs=3) as spool,
            tc.tile_pool(name="opool", bufs=3) as opool,
            tc.tile_pool(name="psum", bufs=4, space="PSUM") as pp,
        ):
            # activation/x/mask pair layout: [128, ko_pair, tile, slot, BT]
            # so a DoubleRow rhs block [128, 2, BT] is contiguous per tile;
            # x and masks arrive pre-packed in this layout (one contiguous
            # DMA per pair).
            XF = K1P * 2 * 2 * BT  # x free elems per partition per pair
            XTF = 2 * BT       # xtail elems per partition per pair

            MF = KO * BT // 2  # mask u32 elems per partition per pair

            gate = {"inst": None}
            cur_xtl = {"tile": None}

            def load_mask(l, pi, queue=None, gate_on=None):
                mtl = mpool.tile([128, KO // 2, 2, 2, BT // 4], u32, tag=f"m{l}", name=f"m{l}_t")
                mi = (queue or nc.gpsimd).dma_start(
                    mtl[:], m_h[l].ap()[:, pi * MF : (pi + 1) * MF]
                )
                if gate_on is not None:
                    tile.add_dep_helper(mi.ins, gate_on, sync=True)
                if gate["inst"] is not None:
                    tile.add_dep_helper(mi.ins, gate["inst"], sync=True)
                return mtl

            def load_x(pi, queue):
                # x is packed tile-major; the two batch tiles of a pair are
                # SEPARATE SBUF tiles (dep tracking is per-tile, so an L1
                # chain starts as soon as its own tile's 512KB half lands).
                # Returns per-tile rhs getters (p -> AP) so pair 0 can use a
                # different tiling without changing hidden_layer.
                xts = []
                for t, tag in ((0, "xta"), (1, "xtb")):
                    xt = xpool.tile([128, K1P, 2, BT], f8, tag=tag, name=tag)
                    di = queue.dma_start(
                        xt[:],
                        xT.ap()[:, pi * XF + t * (XF // 2) : pi * XF + (t + 1) * (XF // 2)],
                    )
                    if gate["inst"] is not None:
                        tile.add_dep_helper(di.ins, gate["inst"], sync=True)
                    xts.append(xt)
                if pi % 2 == 0:
                    # one xtail image per TWO pairs (256KB): small enough
                    # not to delay the next pair's x on the critical queue
                    xtl = xpool.tile([128, 2, 2, BT], f8, tag="xtl", name="xtl")
                    di = queue.dma_start(
                        xtl[:], xtl_h.ap()[:, pi * XTF : (pi + 2) * XTF]
                    )
                    if gate["inst"] is not None:
                        tile.add_dep_helper(di.ins, gate["inst"], sync=True)
                    cur_xtl["tile"] = xtl
                getters = tuple((lambda p, x=x: x[:, p, :, :]) for x in xts)
                return (getters, cur_xtl["tile"], pi % 2), di

            def load_pair(pi):
                # x on the SP HW-DGE queue; weights + masks on the Pool
                # (software DGE) queue - parallel HBM streams, ACT untouched.
                xg, _ = load_x(pi, nc.sync)
                return xg, {l: load_mask(l, pi) for l in DROP_LAYERS}

            # Startup: x pairs 0/1 stream on the Pool software-DGE queue
            # (~300 GB/s) ahead of the hidden weights + first masks; w1 is
            # packed outblock-major as FOUR SEPARATE TILES on the SP queue so
            # the first L1 chain depends on just its own 128KB chunk (dep
            # tracking is per-tile, so chunks into one tile don't help).
            # Warm the PE clock during the fixed ~13us startup window (engine
            # preamble + DMA trigger path) so the first real matmuls run at
            # full p-state instead of ramping through L1.
            warm_w = wpool.tile([128, 2, 128], f8, tag="warmw")
            warm_x = wpool.tile([128, 2, BT], f8, tag="warmx")
            nc.vector.memset(warm_w[:], 0)
            nc.vector.memset(warm_x[:], 0)
            warm_ps = pp.tile([128, 2, BT], f32, tag="ps", name="warm_ps")
            # 9 warms span the whole DMA wait: the PE-idle gap to the first
            # real matmul stays under HAM's ~3.4us re-throttle window even
            # when the x0 stream lands late.
            for _ in range(9):
                nc.tensor.matmul(
                    warm_ps[:, 0, :], lhsT=warm_w[:], rhs=warm_x[:],
                    start=True, stop=True, perf_mode=PM.DoubleRow,
                )

            # Startup: x pairs 0/1 + hidden weights stream on the Pool
            # software-DGE queue (~250 GB/s measured; the HW-DGE queues only
            # do ~78 GB/s); w1 blocks + bias go on the slow-but-parallel SP /
            # ACT HW-DGE queues as four separate tiles so the first L1 chain
            # depends on just its own 128KB block.
            W1F = K1P * 256
            w1n = [
                wpool.tile([128, K1P, 2, 128], f8, tag=f"w1n{n}", name=f"w1n{n}")
                for n in range(KO)
            ]
            bias17_t = wpool.tile([128, 28], f32, tag="bias17")
            nc.sync.dma_start(bias17_t[:], bias17_h.ap())
            w1t_t = wpool.tile([128, 128], f8, tag="w1t")
            nc.sync.dma_start(w1t_t[:], w1t_h.ap())
            nc.sync.dma_start(w1n[0][:], w_h[1].ap()[:, 0:W1F])
            nc.scalar.dma_start(w1n[1][:], w_h[1].ap()[:, W1F : 2 * W1F])
            nc.sync.dma_start(w1n[2][:], w_h[1].ap()[:, 2 * W1F : 3 * W1F])
            nc.scalar.dma_start(w1n[3][:], w_h[1].ap()[:, 3 * W1F : 4 * W1F])
            # Startup pairs 0-3: x + hidden weights + all 12 mask images on
            # the fast Pool SWDGE queue, ordered by first use (4-pair
            # interleave gives the masks until ~42us).  NOTHING with a
            # semaphore wait goes on the ACT/scalar queue - a gated DMA
            # trigger head-of-line-blocks the engine's FIFO and stalls its
            # relus (measured 17us ACT stall).
            # Pair 0 is the DMA critical path: split tile-a into a 256KB
            # chunk-pair half and a 128KB remainder so the very first L1
            # chain starts ~1us earlier (p0/p1 consume the first half, p2
            # lands ~0.5us later, just in time for the 3rd chain slot).
            x0a0 = wpool.tile([128, 2, 2, BT], f8, tag="x0a0")
            nc.gpsimd.dma_start(x0a0[:], xT.ap()[:, 0 : 4 * BT])
            x0a1 = wpool.tile([128, 1, 2, BT], f8, tag="x0a1")
            nc.gpsimd.dma_start(x0a1[:], xT.ap()[:, 4 * BT : 6 * BT])
            x0b = wpool.tile([128, K1P, 2, BT], f8, tag="x0b")
            nc.gpsimd.dma_start(x0b[:], xT.ap()[:, XF // 2 : XF])
            xtl0 = xpool.tile([128, 2, 2, BT], f8, tag="xtl", name="xtl0")
            nc.gpsimd.dma_start(xtl0[:], xtl_h.ap()[:, 0 : 2 * XTF])
            cur_xtl["tile"] = xtl0
            xt0 = (
                (
                    lambda p: x0a0[:, p, :, :] if p < 2 else x0a1[:, 0, :, :],
                    lambda p: x0b[:, p, :, :],
                ),
                xtl0,
                0,
            )
            w_t = {}
            for l in range(2, 8):
                w_t[l] = wpool.tile([128, KO // 2, KO, 2, 128], f8, tag=f"w{l}", name=f"w{l}_t")
            nc.gpsimd.dma_start(w_t[2][:], w_h[2].ap())
            xt1, _ = load_x(1, nc.gpsimd)
            nc.gpsimd.dma_start(w_t[3][:], w_h[3].ap())
            xt2, _ = load_x(2, nc.gpsimd)
            nc.gpsimd.dma_start(w_t[4][:], w_h[4].ap())
            nc.gpsimd.dma_start(w_t[5][:], w_h[5].ap())
            xt3, _ = load_x(3, nc.gpsimd)
            nc.gpsimd.dma_start(w_t[6][:], w_h[6].ap())
            w7_dma = nc.gpsimd.dma_start(w_t[7][:], w_h[7].ap())
            w8_t = wpool.tile([128, KO // 2, 2, 16], f8, tag="w8")
            nc.gpsimd.dma_start(w8_t[:], w8_h.ap())
            b8c_t = wpool.tile([128, 1], f32, tag="b8c")
            nc.gpsimd.dma_start(b8c_t[:], b8c_h.ap())
            mt0, mt1, mt2, mt3 = {}, {}, {}, {}
            for l in DROP_LAYERS:
                mt0[l] = load_mask(l, 0)
                mt1[l] = load_mask(l, 1)
                mt2[l] = load_mask(l, 2)
                mt3[l] = load_mask(l, 3)
            ones10 = wpool.tile([C, C], bf16, tag="ones10")
            nc.vector.memset(ones10[:], 1.0)
            gate["inst"] = w7_dma.ins

            def relu_pair(dst, ps, bias_ap, eng):
                # q8(max(psum + s_l*b_l, 0)) for both tiles, PSUM -> fp8
                if eng == "A":
                    nc.scalar.activation(dst, ps, AF.Relu, bias=bias_ap)
                else:
                    nc.vector.tensor_scalar(dst, ps, bias_ap, 0.0, ALU.add, ALU.max)

            def layer1(src, eng):
                # L1 contracts 768 features via 3 DoubleRow chunks per block
                # plus a K=16 leftover matmul per block.  The four leftover
                # matmuls of a kp-group run CONCURRENTLY on distinct 32-row
                # quadrants (tile_position row packing): ~2 slot times
                # instead of 4, saving 4 x 216ns per pair vs padding to 1024.
                getters, xtl, sub = src
                hn = hpool.tile([128, KO // 2, 2, 2, BT], f8, tag="h", name="h")
                for kp in range(2):
                    ns = (2 * kp, 2 * kp + 1)
                    pss = {n: pp.tile([128, 2, BT], f32, tag="ps", name="ps") for n in ns}
                    for n in ns:
                        for t in range(2):
                            for p in range(K1P):
                                nc.tensor.matmul(
                                    pss[n][:, t, :],
                                    lhsT=w1n[n][:, p, :, :],
                                    rhs=getters[t](p),
                                    start=(p == 0),
                                    stop=False,
                                    perf_mode=PM.DoubleRow,
                                    skip_group_check=True,
                                )
                    # K=16 leftover matmuls close both blocks' groups as a
                    # row-quadrant-packed burst (two concurrent per round).
                    for t in range(2):
                        for n in ns:
                            nc.tensor.matmul(
                                pss[n][:, t, :],
                                lhsT=w1t_t[32 * n : 32 * n + K1_TAIL, :],
                                rhs=xtl[32 * n : 32 * n + K1_TAIL, sub, t, :],
                                start=False,
                                stop=True,
                                tile_position=(32 * n, 0),
                                skip_group_check=True,
                            )
                    for n in ns:
                        relu_pair(
                            hn[:, kp, :, n % 2, :],
                            pss[n][:],
                            bias17_t[:, n : n + 1],
                            eng[n],
                        )
                return hn

            def hidden_layer(l, src, mt, parity=0):
                pairs_in = K1P if l == 1 else KO // 2
                eng = RELU_ENG[l]
                if isinstance(eng, tuple):
                    eng = eng[parity]
                if l == 1:
                    return layer1(src, eng), None
                hn = hpool.tile([128, KO // 2, 2, 2, BT], f8, tag="h", name="h")
                # Drop layers process block 2 (the DVE relu) LAST so the
                # single whole-layer AND directly follows it on DVE and its
                # PSUM-ring slot is the last one the next pair-layer
                # overwrites.
                order = (0, 1, 3, 2) if l in DROP_LAYERS else range(KO)
                for n in order:
                    ps = pp.tile([128, 2, BT], f32, tag="ps", name="ps")
                    for t in range(2):
                        for p in range(pairs_in):
                            if l == 1:
                                lhsT = w1n[n][:, p, :, :]
                                rhs = src[t](p)  # x: per-tile rhs getters
                            else:
                                lhsT = w_t[l][:, p, n, :, :]
                                rhs = src[:, p, t, :, :]
                            nc.tensor.matmul(
                                ps[:, t, :],
                                lhsT=lhsT,
                                rhs=rhs,
                                start=(p == 0),
                                stop=(p == pairs_in - 1),
                                perf_mode=PM.DoubleRow,
                            )
                    relu_pair(
                        hn[:, n // 2, :, n % 2, :],
                        ps[:],
                        bias17_t[:, (l - 1) * 4 + n : (l - 1) * 4 + n + 1],
                        eng[n],
                    )
                if l in DROP_LAYERS:
                    # The AND's emission is DEFERRED by the caller to after
                    # the NEXT pair's relus: a whole-layer AND sitting ahead
                    # of the following window's relus in the in-order DVE
                    # FIFO delays them and stalls the PE via the PSUM ring.
                    def and_fn(hn=hn, l=l, mt=mt):
                        d32 = hn[:].bitcast(u32)
                        nc.vector.tensor_tensor(
                            d32, d32, mt[l][:, :, :, :, :], ALU.bitwise_and
                        )
                    return hn, and_fn
                return hn, None

            pending = []

            def final_head(h, pi):
                # layer 8 (512->10 padded 16) for both tiles + exp -> bf16.
                ps8 = pp.tile([128, 2, BT], f32, tag="ps", name="ps8")
                for t in range(2):
                    for p in range(KO // 2):
                        nc.tensor.matmul(
                            ps8[:16, t, :],
                            lhsT=w8_t[:, p, :, :],
                            rhs=h[:, p, t, :, :],
                            start=(p == 0),
                            stop=(p == KO // 2 - 1),
                            perf_mode=PM.DoubleRow,
                        )
                ex = spool.tile([C, 2, BT], bf16, tag="ex", name="ex")
                nc.scalar.activation(
                    ex[:], ps8[:C, :, :], AF.Exp, bias=b8c_t[:C, 0:1], scale=float(g8)
                )
                pending.append((ex, pi))

            def final_last(h, pi):
                # the very last pair: per-tile staggered tail so the two
                # half-chains (exp -> sum -> recip -> mult -> store) pipeline
                # instead of draining serially after the final matmul.
                bs = pi * PW
                ps8 = pp.tile([128, 2, BT], f32, tag="ps", name="ps8")
                for t in range(2):
                    for p in range(KO // 2):
                        nc.tensor.matmul(
                            ps8[:16, t, :],
                            lhsT=w8_t[:, p, :, :],
                            rhs=h[:, p, t, :, :],
                            start=(p == 0),
                            stop=(p == KO // 2 - 1),
                            perf_mode=PM.DoubleRow,
                        )
                    # exp for tile t issues as soon as its chains finish
                    if t == 0:
                        exl = spool.tile([C, 2, BT], bf16, tag="ex", name="exl")
                    nc.scalar.activation(
                        exl[:, t, :], ps8[:C, t, :], AF.Exp,
                        bias=b8c_t[:C, 0:1], scale=float(g8),
                    )
                ps_s = pp.tile([128, 2, BT], f32, tag="ps", name="ps_sl")
                rs = spool.tile([C, 2, BT], f32, tag="rs", name="rsl")
                ot = opool.tile([C, 2, BT], f32, tag="ot", name="otl")
                for t in range(2):
                    nc.tensor.matmul(
                        ps_s[:C, t, :], lhsT=ones10[:], rhs=exl[:, t, :],
                        start=True, stop=True,
                    )
                    nc.vector.reciprocal_approx_fast(rs[:, t, :], ps_s[:C, t, :])
                    eng = nc.gpsimd if t == 0 else nc.vector
                    eng.tensor_tensor(ot[:, t, :], exl[:, t, :], rs[:, t, :], ALU.mult)
                    nc.sync.dma_start(
                        y_h.ap()[:, bs + t * BT : bs + (t + 1) * BT], ot[:, t, :]
                    )

            def flush_tail(last=False):
                # class-sum matmul + reciprocal + multiply + store; issued
                # late so PE never waits on the exp round-trip, one tail per
                # layer-1 slot so the shared ps8 ring never stalls PE.
                if not pending:
                    return
                ex, pi = pending.pop(0)
                bs = pi * PW
                ps_s = pp.tile([128, 2, BT], f32, tag="ps", name="ps_s")
                for t in range(2):
                    nc.tensor.matmul(
                        ps_s[:C, t, :], lhsT=ones10[:], rhs=ex[:, t, :],
                        start=True, stop=True,
                    )
                rs = spool.tile([C, 2, BT], f32, tag="rs", name="rs")
                nc.vector.reciprocal_approx_fast(rs[:], ps_s[:C, :, :])
                ot = opool.tile([C, 2, BT], f32, tag="ot", name="ot")
                # multiply on Pool (idle) so only the reciprocal loads DVE;
                # the very last tail overlaps its sibling via DVE.
                eng = nc.vector if last == 2 else nc.gpsimd
                eng.tensor_tensor(ot[:], ex[:], rs[:], ALU.mult)
                nc.sync.dma_start(y_h.ap()[:, bs : bs + PW], ot[:])

            def process_group(prs, is_last=False):
                # FOUR pairs interleaved at layer granularity: every
                # cross-layer dependency (relu/AND chain -> next layer's
                # matmuls) gets three sibling pair-layers (~10us) of slack,
                # so transient ACT/DVE backlogs never stall the PE.
                hs = [p[0] for p in prs]
                n_p = len(prs)
                pend_and = [None]

                def step(l, j):
                    hs[j], afn = hidden_layer(l, hs[j], prs[j][1], parity=j % 2)
                    # previous pair's dropout AND lands AFTER this window's
                    # relus in the DVE queue (its consumer is 3-4 windows
                    # away), so it never delays the critical relu chain.
                    if pend_and[0] is not None:
                        pend_and[0]()
                    pend_and[0] = afn

                for l in range(1, 7):
                    for j in range(n_p):
                        step(l, j)
                        # previous group's softmax tails are all flushed in
                        # the roomy L1 step (6.9us windows): a DVE reciprocal
                        # in a 3.46us hidden window overloads DVE right after
                        # the drop step's AND backlog.
                        if l == 1:
                            flush_tail()
                for j in range(n_p):
                    step(7, j)
                    if is_last and j == n_p - 1:
                        # drain the second-to-last tail while this pair's
                        # L8+exp run, then the very last tail.
                        flush_tail(last=2)
                        final_last(hs[j], prs[j][2])
                    else:
                        # L8-j right after L7-j: its tail chain overlaps the
                        # later pairs' L7/L8 instead of draining at the end.
                        final_head(hs[j], prs[j][2])
                        if is_last and j >= 1:
                            flush_tail(last=(1 if j == n_p - 2 else 0))

            process_group(
                [(xt0, mt0, 0), (xt1, mt1, 1), (xt2, mt2, 2), (xt3, mt3, 3)]
            )
            grp = []
            for pi in range(4, npair):
                xg, mg = load_pair(pi)
                grp.append((xg, mg, pi))
            process_group(grp, is_last=True)
            flush_tail(last=2)

    nc.compile()
    return nc


def host_prepare(inputs: dict) -> tuple[dict, dict, float]:
    """Calibrate fp8 scales, quantize weights, compute masks, shard x.

    Returns (shared_inputs, per_core_varying, g8) where per_core_varying maps
    name -> list of 8 per-core arrays.
    """
    import jax

    x = np.asarray(inputs["x"], dtype=np.float32)
    W = {i: np.asarray(inputs[f"W{i}"], dtype=np.float32) for i in range(1, 9)}
    b = {i: np.asarray(inputs[f"b{i}"], dtype=np.float32) for i in range(1, 9)}

    # Dropout masks — bit-exact replication of the reference's PRNG stream.
    cpu = jax.devices("cpu")[0]
    with jax.default_device(cpu):
        dk = jax.random.split(jax.random.key(42), 3)
        keeps = {
            l: np.asarray(
                jax.random.bernoulli(dk[i], KEEP[l], (BATCH, H)), dtype=np.float32
            )
            for i, l in enumerate(DROP_LAYERS)
        }

    # Fold 1/(1-p) into the next layer's weights.
    Wf = dict(W)
    for l in DROP_LAYERS:
        Wf[l + 1] = (W[l + 1] / np.float32(KEEP[l])).astype(np.float32)

    # ---- calibration: fp32 forward on 2048 rows to pick pow2 scales ----
    def pow2(v):
        return np.float32(2.0 ** np.round(np.log2(v)))

    ncal = 2048
    h = x[:ncal]
    s = {0: pow2(8.0 / np.sqrt(np.mean(h**2)))}
    for l in range(1, 8):
        h = np.maximum(h @ Wf[l] + b[l], 0.0)
        if l in DROP_LAYERS:
            h = h * keeps[l][:ncal]
        s[l] = pow2(8.0 / max(np.sqrt(np.mean(h**2)), 1e-6))
    ws8 = pow2(8.0 / np.sqrt(np.mean(Wf[8] ** 2)))
    g8 = float(1.0 / (s[7] * ws8))

    # ---- quantize weights: layer l scale is exactly s_l / s_{l-1} ----
    def pack_dual(Wq, ncol):
        """[pairs*2*128, n_blocks*ncol] -> [128, pairs*n_blocks*2*ncol] with
        each DoubleRow lhsT block [128, 2, ncol] contiguous."""
        K, N = Wq.shape
        pairs, n_blocks = K // 256, N // ncol
        arr = Wq.reshape(pairs, 2, 128, n_blocks, ncol).transpose(2, 0, 3, 1, 4)
        return np.ascontiguousarray(arr.reshape(128, pairs * n_blocks * 2 * ncol))

    def pack_dual_nmajor(Wq, ncol):
        """Like pack_dual but outblock-major: [128, n_blocks*pairs*2*ncol]."""
        K, N = Wq.shape
        pairs, n_blocks = K // 256, N // ncol
        arr = Wq.reshape(pairs, 2, 128, n_blocks, ncol).transpose(2, 3, 0, 1, 4)
        return np.ascontiguousarray(arr.reshape(128, -1))

    W8q = {}
    # w1: 768 features as 3 DoubleRow chunk-pairs; the 16 leftover features
    # (768..783) as a separate [16,128]-per-block tail, replicated into the
    # four 32-row quadrant groups for row-packed K=16 matmuls.
    W1q = (Wf[1] * (s[1] / s[0])).astype(E4)
    W8q[1] = pack_dual_nmajor(np.ascontiguousarray(W1q[:K1_MAIN]), 128)
    w1tail = np.zeros((128, 128), dtype=E4)
    for n in range(KO):
        w1tail[32 * n : 32 * n + K1_TAIL, :] = W1q[K1_MAIN:, 128 * n : 128 * (n + 1)]
    for l in range(2, 8):
        W8q[l] = pack_dual((Wf[l] * (s[l] / s[l - 1])).astype(E4), 128)
    W8p = np.zeros((H, 16), dtype=np.float32)
    W8p[:, :C] = Wf[8] * ws8
    W8q[8] = pack_dual(W8p.astype(E4), 16)

    # biases: s_l * b_l, packed [128, 4] per layer
    bias17 = np.empty((128, 28), dtype=np.float32)
    for l in range(1, 8):
        bias17[:, (l - 1) * 4 : l * 4] = (s[l] * b[l]).reshape(4, 128).T
    b8c = np.zeros((128, 1), dtype=np.float32)
    b8c[:C, 0] = b[8]

    # x: quantize, transpose; 768 features DR-packed + 16-feature tail
    # replicated across the four row-quadrant partition groups
    xTq = (x.T * s[0]).astype(E4)
    xTp = np.ascontiguousarray(xTq[:K1_MAIN])
    xtail = np.zeros((128, BATCH), dtype=E4)
    for q in range(KO):
        xtail[32 * q : 32 * q + K1_TAIL, :] = xTq[K1_MAIN:]

    def pack_act(a):
        """[F, B_CORE] feature-major -> [128, npair*F/128*2*BT] in the SBUF
        pair layout [p, pair, ko_pair, tile, slot, BT]."""
        F, Bc = a.shape
        v = a.reshape(F // 256, 2, 128, Bc // PW, 2, BT)  # [pr, sl, p, pair, t, b]
        v = v.transpose(2, 3, 0, 4, 1, 5)                 # [p, pair, pr, t, sl, b]
        return np.ascontiguousarray(v.reshape(128, -1))

    def pack_x(a):
        """Tile-major variant for x: [p, pair, tile, ko_pair, slot, BT] so
        each batch tile's half is one contiguous DMA."""
        F, Bc = a.shape
        v = a.reshape(F // 256, 2, 128, Bc // PW, 2, BT)  # [pr, sl, p, pair, t, b]
        v = v.transpose(2, 3, 4, 0, 1, 5)                 # [p, pair, t, pr, sl, b]
        return np.ascontiguousarray(v.reshape(128, -1))

    shared = {
        "w1": W8q[1],
        "w1t": w1tail,
        "w8": W8q[8],
        "bias17": bias17,
        "b8c": b8c,
    }
    for l in range(2, 8):
        shared[f"w{l}"] = W8q[l]

    per_core = {"xT": [], "xtl": [], "m2": [], "m4": [], "m6": []}
    mT = {
        l: np.where(keeps[l].T != 0, 255, 0).astype(np.uint8) for l in DROP_LAYERS
    }
    for c in range(N_CORES):
        sl = slice(c * B_CORE, (c + 1) * B_CORE)
        per_core["xT"].append(pack_x(xTp[:, sl]))
        # xtail pair layout: [128, pair, tile, BT]
        xt = xtail[:, sl].reshape(128, B_CORE // PW, 2, BT)
        per_core["xtl"].append(np.ascontiguousarray(xt.reshape(128, -1)))
        for l in DROP_LAYERS:
            per_core[f"m{l}"].append(pack_act(mT[l][:, sl]).view(np.uint32))
    return shared, per_core, g8


def run_hw(inputs: dict, trace: bool = False):
    from concourse import bass_utils

    shared, per_core, g8 = host_prepare(inputs)
    nc = build_bass(B_CORE, g8)
    in_maps = [
        {**shared, **{k: v[c] for k, v in per_core.items()}} for c in range(N_CORES)
    ]
    res = bass_utils.run_bass_kernel_spmd(
        nc, in_maps, core_ids=list(range(N_CORES)), trace=trace
    )
    out = np.concatenate([np.ascontiguousarray(r["yT"].T) for r in res.results], axis=0)
    return out.astype(np.float32), res


def kernel(**inputs) -> np.ndarray:
    return run_hw(inputs, trace=False)[0]



# revision 65
# speedup vs baseline: 1.0076x; 1.0076x over previous
"""Trainium2 Bass kernel for an 8-layer dense MLP (784->512x6->10) + softmax.

Strategy (hardcoded for batch=65536, 8 NeuronCores, pure data parallel):
  - Each core handles 8192 rows of the batch; weights replicated.
  - All matmuls run in fp8-e4m3 with MatmulPerfMode.DoubleRow (256-feature
    contraction per instruction; one 512-px matmul issues every ~216ns =
    the fp8 peak).  PE is the bottleneck: everything else is scheduled to
    keep its 216ns cadence unbroken.
  - Layer 1 contracts 768 of the 784 input features with 3 DoubleRow chunks
    and handles the 16 leftover features as K=16 normal-mode matmuls packed
    two-concurrent onto 32-row array quadrants via tile_position - saving
    most of the pad-to-1024 waste.
  - Quantization scales: per-layer power-of-2 activation scales s_l picked by
    a 2048-row fp32 calibration forward on host; weight scale for layer l is
    exactly s_l/s_{l-1}, so PSUM already carries s_l * preactivation and the
    PSUM->fp8 step is a single fused  q8(max(psum + s_l*b_l, 0))  on either
    ACT (activation, bias=) or DVE (tensor_scalar add,max).
  - FOUR pairs (of two BT=512 batch tiles) are in flight, interleaved at
    layer granularity: every relu/AND -> next-layer dependency gets three
    sibling pair-layer windows (~10us) of slack.  Drop layers alternate the
    relu split (3 ACT/1 DVE vs 2/2+1) per pair parity so neither post-op
    engine runs saturated for four consecutive windows.
  - Dropout masks (jax threefry, key 42) are bit-exactly precomputed on host
    and shipped as u32 {0x00,0xFF}-byte words; applied as ONE whole-layer
    bitwise-AND on DVE whose emission is deferred past the next pair's
    relus (in-order DVE FIFO: an early AND delays the critical relus).
    1/(1-p) is folded into the next layer's weights on host.
  - Softmax: exp on ACT -> bf16 (scale=g8 dequant, bias=b8), replicated
    class-sum via a [10,10] all-ones matmul on PE, reciprocal_approx_fast on
    DVE + multiply on Pool.  Tails are deferred and flushed in the next
    group's roomy L1 windows; the final group drains them staggered between
    the last pairs' L7/L8 so the kernel tail stays short.
  - All PSUM flows through one 4-slot ring of [128,2,BT] two-bank pair
    tiles (8 banks total).  Startup: x + hidden weights + masks stream on
    the Pool software-DGE queue (~250 GB/s measured; HW-DGE queues only
    ~78 GB/s) in first-use order; only the small w1/bias tiles ride the
    HW-DGE queues, and nothing with a semaphore wait is ever placed on a
    compute engine's queue (a gated DMA trigger head-of-line-blocks it).
"""

import numpy as np
import ml_dtypes

E4 = ml_dtypes.float8_e4m3

BATCH = 65536
D_IN = 784
K1_MAIN = 768              # input features contracted via 3 DoubleRow pairs
K1P = K1_MAIN // 256       # = 3 DR chunk-pairs for layer 1
K1_TAIL = D_IN - K1_MAIN   # = 16 leftover features, row-quadrant packed
H = 512
KO = H // 128              # 4 feature chunks for hidden layers
C = 10
N_CORES = 8
B_CORE = BATCH // N_CORES  # 8192
BT = 512                   # batch tile (matmul moving free dim)
PW = 2 * BT                # pair width

DROP_LAYERS = (2, 4, 6)    # dropout applied to these layers' outputs
KEEP = {2: 0.8, 4: 0.7, 6: 0.5}


def build_bass(b_core: int, g8: float):
    """Build the Bass module for one core processing b_core batch rows."""
    import concourse.mybir as mybir
    import concourse.tile as tile
    from concourse import bacc

    f32 = mybir.dt.float32
    f8 = mybir.dt.float8e4
    bf16 = mybir.dt.bfloat16
    AF = mybir.ActivationFunctionType
    ALU = mybir.AluOpType
    PM = mybir.MatmulPerfMode

    npair = b_core // PW

    nc = bacc.Bacc("TRN2", target_bir_lowering=False, debug=False)

    # Weights are pre-packed on host so every DoubleRow lhsT block
    # [128, 2, ncol] is contiguous in SBUF (s3_lw_dual_fp8_restrictions):
    # layer l image is [128, pairs, KO, 2, 128] flattened to 2D.
    # x and masks are pre-packed into the SBUF pair layout
    # [p, pair, ko_pair, tile, slot, BT] flattened to 2D so each pair loads
    # with one fully-contiguous DMA.
    xT = nc.dram_tensor("xT", [128, npair * K1P * 2 * 2 * BT], f8, kind="ExternalInput")
    xtl_h = nc.dram_tensor("xtl", [128, npair * 2 * BT], f8, kind="ExternalInput")
    w_h = {1: nc.dram_tensor("w1", [128, K1P * KO * 256], f8, kind="ExternalInput")}
    w1t_h = nc.dram_tensor("w1t", [128, 128], f8, kind="ExternalInput")
    for l in range(2, 8):
        w_h[l] = nc.dram_tensor(f"w{l}", [128, (KO // 2) * KO * 256], f8, kind="ExternalInput")
    w8_h = nc.dram_tensor("w8", [128, (KO // 2) * 2 * 16], f8, kind="ExternalInput")
    bias17_h = nc.dram_tensor("bias17", [128, 28], f32, kind="ExternalInput")
    b8c_h = nc.dram_tensor("b8c", [128, 1], f32, kind="ExternalInput")
    u32 = mybir.dt.uint32
    # masks ship as uint32 {0x00.., 0xFF..} bytes covering four fp8 lanes
    # each: the dropout multiply is a bitwise AND on u32 views (DVE only —
    # Pool has no bitwise ops and DVE bitwise requires 32-bit).
    m_h = {
        l: nc.dram_tensor(f"m{l}", [128, npair * KO * BT // 2], u32, kind="ExternalInput")
        for l in DROP_LAYERS
    }
    y_h = nc.dram_tensor("yT", [C, b_core], f32, kind="ExternalOutput")

    # Per-layer relu/quantize engine map (A=ACT, D=DVE), one char per output
    # block n=0..3.  Drop layers alternate the 3:1 split per pair parity so
    # neither engine runs >99% for four consecutive windows (the mask-AND
    # rides DVE); other layers split evenly.  Block 0 is always ACT-first:
    # the next pair-layer's first chain WAR-waits on it via the PSUM ring.
    RELU_ENG = {
        1: "ADAD", 2: ("AADA", "ADDA"), 3: "ADAD", 4: ("AADA", "ADDA"),
        5: "ADAD", 6: ("AADA", "ADDA"), 7: "ADAD",
    }

    with tile.TileContext(nc) as tc:
        with (
            tc.tile_pool(name="wpool", bufs=1) as wpool,
            tc.tile_pool(name="xpool", bufs=5) as xpool,
            tc.tile_pool(name="hpool", bufs=8) as hpool,
            tc.tile_pool(name="mpool", bufs=5) as mpool,
            tc.tile_pool(name="spool", bufs=3) as spool,
            tc.tile_pool(name="opool", bufs=3) as opool,
            tc.tile_pool(name="psum", bufs=4, space="PSUM") as pp,
        ):
            # activation/x/mask pair layout: [128, ko_pair, tile, slot, BT]
            # so a DoubleRow rhs block [128, 2, BT] is contiguous per tile;
            # x and masks arrive pre-packed in this layout (one contiguous
            # DMA per pair).
            XF = K1P * 2 * 2 * BT  # x free elems per partition per pair
            XTF = 2 * BT       # xtail elems per partition per pair

            MF = KO * BT // 2  # mask u32 elems per partition per pair

            gate = {"inst": None}
            cur_xtl = {"tile": None}

            def load_mask(l, pi, queue=None, gate_on=None):
                mtl = mpool.tile([128, KO // 2, 2, 2, BT // 4], u32, tag=f"m{l}", name=f"m{l}_t")
                mi = (queue or nc.gpsimd).dma_start(
                    mtl[:], m_h[l].ap()[:, pi * MF : (pi + 1) * MF]
                )
                if gate_on is not None:
                    tile.add_dep_helper(mi.ins, gate_on, sync=True)
                if gate["inst"] is not None:
                    tile.add_dep_helper(mi.ins, gate["inst"], sync=True)
                return mtl

            def load_x(pi, queue):
                # x is packed tile-major; the two batch tiles of a pair are
                # SEPARATE SBUF tiles (dep tracking is per-tile, so an L1
                # chain starts as soon as its own tile's 512KB half lands).
                # Returns per-tile rhs getters (p -> AP) so pair 0 can use a
                # different tiling without changing hidden_layer.
                xts = []
                for t, tag in ((0, "xta"), (1, "xtb")):
                    xt = xpool.tile([128, K1P, 2, BT], f8, tag=tag, name=tag)
                    di = queue.dma_start(
                        xt[:],
                        xT.ap()[:, pi * XF + t * (XF // 2) : pi * XF + (t + 1) * (XF // 2)],
                    )
                    if gate["inst"] is not None:
                        tile.add_dep_helper(di.ins, gate["inst"], sync=True)
                    xts.append(xt)
                if pi % 2 == 0:
                    # one xtail image per TWO pairs (256KB): small enough
                    # not to delay the next pair's x on the critical queue
                    xtl = xpool.tile([128, 2, 2, BT], f8, tag="xtl", name="xtl")
                    di = queue.dma_start(
                        xtl[:], xtl_h.ap()[:, pi * XTF : (pi + 2) * XTF]
                    )
                    if gate["inst"] is not None:
                        tile.add_dep_helper(di.ins, gate["inst"], sync=True)
                    cur_xtl["tile"] = xtl
                getters = tuple((lambda p, x=x: x[:, p, :, :]) for x in xts)
                return (getters, cur_xtl["tile"], pi % 2), di

            def load_pair(pi):
                # x on the SP HW-DGE queue; weights + masks on the Pool
                # (software DGE) queue - parallel HBM streams, ACT untouched.
                xg, _ = load_x(pi, nc.sync)
                return xg, {l: load_mask(l, pi) for l in DROP_LAYERS}

            # Startup: x pairs 0/1 stream on the Pool software-DGE queue
            # (~300 GB/s) ahead of the hidden weights + first masks; w1 is
            # packed outblock-major as FOUR SEPARATE TILES on the SP queue so
            # the first L1 chain depends on just its own 128KB chunk (dep
            # tracking is per-tile, so chunks into one tile don't help).
            # Warm the PE clock during the fixed ~13us startup window (engine
            # preamble + DMA trigger path) so the first real matmuls run at
            # full p-state instead of ramping through L1.
            warm_w = wpool.tile([128, 2, 128], f8, tag="warmw")
            warm_x = wpool.tile([128, 2, BT], f8, tag="warmx")
            nc.vector.memset(warm_w[:], 0)
            nc.vector.memset(warm_x[:], 0)
            warm_ps = pp.tile([128, 2, BT], f32, tag="ps", name="warm_ps")
            for _ in range(6):
                nc.tensor.matmul(
                    warm_ps[:, 0, :], lhsT=warm_w[:], rhs=warm_x[:],
                    start=True, stop=True, perf_mode=PM.DoubleRow,
                )

            # Startup: x pairs 0/1 + hidden weights stream on the Pool
            # software-DGE queue (~250 GB/s measured; the HW-DGE queues only
            # do ~78 GB/s); w1 blocks + bias go on the slow-but-parallel SP /
            # ACT HW-DGE queues as four separate tiles so the first L1 chain
            # depends on just its own 128KB block.
            W1F = K1P * 256
            w1n = [
                wpool.tile([128, K1P, 2, 128], f8, tag=f"w1n{n}", name=f"w1n{n}")
                for n in range(KO)
            ]
            bias17_t = wpool.tile([128, 28], f32, tag="bias17")
            nc.sync.dma_start(bias17_t[:], bias17_h.ap())
            w1t_t = wpool.tile([128, 128], f8, tag="w1t")
            nc.sync.dma_start(w1t_t[:], w1t_h.ap())
            nc.sync.dma_start(w1n[0][:], w_h[1].ap()[:, 0:W1F])
            nc.scalar.dma_start(w1n[1][:], w_h[1].ap()[:, W1F : 2 * W1F])
            nc.sync.dma_start(w1n[2][:], w_h[1].ap()[:, 2 * W1F : 3 * W1F])
            nc.scalar.dma_start(w1n[3][:], w_h[1].ap()[:, 3 * W1F : 4 * W1F])
            # Startup pairs 0-3: x + hidden weights + all 12 mask images on
            # the fast Pool SWDGE queue, ordered by first use (4-pair
            # interleave gives the masks until ~42us).  NOTHING with a
            # semaphore wait goes on the ACT/scalar queue - a gated DMA
            # trigger head-of-line-blocks the engine's FIFO and stalls its
            # relus (measured 17us ACT stall).
            # Pair 0 is the DMA critical path: split tile-a into a 256KB
            # chunk-pair half and a 128KB remainder so the very first L1
            # chain starts ~1us earlier (p0/p1 consume the first half, p2
            # lands ~0.5us later, just in time for the 3rd chain slot).
            x0a0 = wpool.tile([128, 2, 2, BT], f8, tag="x0a0")
            nc.gpsimd.dma_start(x0a0[:], xT.ap()[:, 0 : 4 * BT])
            x0a1 = wpool.tile([128, 1, 2, BT], f8, tag="x0a1")
            nc.gpsimd.dma_start(x0a1[:], xT.ap()[:, 4 * BT : 6 * BT])
            x0b = wpool.tile([128, K1P, 2, BT], f8, tag="x0b")
            nc.gpsimd.dma_start(x0b[:], xT.ap()[:, XF // 2 : XF])
            xtl0 = xpool.tile([128, 2, 2, BT], f8, tag="xtl", name="xtl0")
            nc.gpsimd.dma_start(xtl0[:], xtl_h.ap()[:, 0 : 2 * XTF])
            cur_xtl["tile"] = xtl0
            xt0 = (
                (
                    lambda p: x0a0[:, p, :, :] if p < 2 else x0a1[:, 0, :, :],
                    lambda p: x0b[:, p, :, :],
                ),
                xtl0,
                0,
            )
            w_t = {}
            for l in range(2, 8):
                w_t[l] = wpool.tile([128, KO // 2, KO, 2, 128], f8, tag=f"w{l}", name=f"w{l}_t")
            nc.gpsimd.dma_start(w_t[2][:], w_h[2].ap())
            xt1, _ = load_x(1, nc.gpsimd)
            nc.gpsimd.dma_start(w_t[3][:], w_h[3].ap())
            xt2, _ = load_x(2, nc.gpsimd)
            nc.gpsimd.dma_start(w_t[4][:], w_h[4].ap())
            nc.gpsimd.dma_start(w_t[5][:], w_h[5].ap())
            xt3, _ = load_x(3, nc.gpsimd)
            nc.gpsimd.dma_start(w_t[6][:], w_h[6].ap())
            w7_dma = nc.gpsimd.dma_start(w_t[7][:], w_h[7].ap())
            w8_t = wpool.tile([128, KO // 2, 2, 16], f8, tag="w8")
            nc.gpsimd.dma_start(w8_t[:], w8_h.ap())
            b8c_t = wpool.tile([128, 1], f32, tag="b8c")
            nc.gpsimd.dma_start(b8c_t[:], b8c_h.ap())
            mt0, mt1, mt2, mt3 = {}, {}, {}, {}
            for l in DROP_LAYERS:
                mt0[l] = load_mask(l, 0)
                mt1[l] = load_mask(l, 1)
                mt2[l] = load_mask(l, 2)
                mt3[l] = load_mask(l, 3)
            ones10 = wpool.tile([C, C], bf16, tag="ones10")
            nc.vector.memset(ones10[:], 1.0)
            gate["inst"] = w7_dma.ins

            def relu_pair(dst, ps, bias_ap, eng):
                # q8(max(psum + s_l*b_l, 0)) for both tiles, PSUM -> fp8
                if eng == "A":
                    nc.scalar.activation(dst, ps, AF.Relu, bias=bias_ap)
                else:
                    nc.vector.tensor_scalar(dst, ps, bias_ap, 0.0, ALU.add, ALU.max)

            def layer1(src, eng):
                # L1 contracts 768 features via 3 DoubleRow chunks per block
                # plus a K=16 leftover matmul per block.  The four leftover
                # matmuls of a kp-group run CONCURRENTLY on distinct 32-row
                # quadrants (tile_position row packing): ~2 slot times
                # instead of 4, saving 4 x 216ns per pair vs padding to 1024.
                getters, xtl, sub = src
                hn = hpool.tile([128, KO // 2, 2, 2, BT], f8, tag="h", name="h")
                for kp in range(2):
                    ns = (2 * kp, 2 * kp + 1)
                    pss = {n: pp.tile([128, 2, BT], f32, tag="ps", name="ps") for n in ns}
                    for n in ns:
                        for t in range(2):
                            for p in range(K1P):
                                nc.tensor.matmul(
                                    pss[n][:, t, :],
                                    lhsT=w1n[n][:, p, :, :],
                                    rhs=getters[t](p),
                                    start=(p == 0),
                                    stop=False,
                                    perf_mode=PM.DoubleRow,
                                    skip_group_check=True,
                                )
                    # K=16 leftover matmuls close both blocks' groups as a
                    # row-quadrant-packed burst (two concurrent per round).
                    for t in range(2):
                        for n in ns:
                            nc.tensor.matmul(
                                pss[n][:, t, :],
                                lhsT=w1t_t[32 * n : 32 * n + K1_TAIL, :],
                                rhs=xtl[32 * n : 32 * n + K1_TAIL, sub, t, :],
                                start=False,
                                stop=True,
                                tile_position=(32 * n, 0),
                                skip_group_check=True,
                            )
                    for n in ns:
                        relu_pair(
                            hn[:, kp, :, n % 2, :],
                            pss[n][:],
                            bias17_t[:, n : n + 1],
                            eng[n],
                        )
                return hn

            def hidden_layer(l, src, mt, parity=0):
                pairs_in = K1P if l == 1 else KO // 2
                eng = RELU_ENG[l]
                if isinstance(eng, tuple):
                    eng = eng[parity]
                if l == 1:
                    return layer1(src, eng), None
                hn = hpool.tile([128, KO // 2, 2, 2, BT], f8, tag="h", name="h")
                # Drop layers process block 2 (the DVE relu) LAST so the
                # single whole-layer AND directly follows it on DVE and its
                # PSUM-ring slot is the last one the next pair-layer
                # overwrites.
                order = (0, 1, 3, 2) if l in DROP_LAYERS else range(KO)
                for n in order:
                    ps = pp.tile([128, 2, BT], f32, tag="ps", name="ps")
                    for t in range(2):
                        for p in range(pairs_in):
                            if l == 1:
                                lhsT = w1n[n][:, p, :, :]
                                rhs = src[t](p)  # x: per-tile rhs getters
                            else:
                                lhsT = w_t[l][:, p, n, :, :]
                                rhs = src[:, p, t, :, :]
                            nc.tensor.matmul(
                                ps[:, t, :],
                                lhsT=lhsT,
                                rhs=rhs,
                                start=(p == 0),
                                stop=(p == pairs_in - 1),
                                perf_mode=PM.DoubleRow,
                            )
                    relu_pair(
                        hn[:, n // 2, :, n % 2, :],
                        ps[:],
                        bias17_t[:, (l - 1) * 4 + n : (l - 1) * 4 + n + 1],
                        eng[n],
                    )
                if l in DROP_LAYERS:
                    # The AND's emission is DEFERRED by the caller to after
                    # the NEXT pair's relus: a whole-layer AND sitting ahead
                    # of the following window's relus in the in-order DVE
                    # FIFO delays them and stalls the PE via the PSUM ring.
                    def and_fn(hn=hn, l=l, mt=mt):
                        d32 = hn[:].bitcast(u32)
                        nc.vector.tensor_tensor(
                            d32, d32, mt[l][:, :, :, :, :], ALU.bitwise_and
                        )
                    return hn, and_fn
                return hn, None

            pending = []

            def final_head(h, pi):
                # layer 8 (512->10 padded 16) for both tiles + exp -> bf16.
                ps8 = pp.tile([128, 2, BT], f32, tag="ps", name="ps8")
                for t in range(2):
                    for p in range(KO // 2):
                        nc.tensor.matmul(
                            ps8[:16, t, :],
                            lhsT=w8_t[:, p, :, :],
                            rhs=h[:, p, t, :, :],
                            start=(p == 0),
                            stop=(p == KO // 2 - 1),
                            perf_mode=PM.DoubleRow,
                        )
                ex = spool.tile([C, 2, BT], bf16, tag="ex", name="ex")
                nc.scalar.activation(
                    ex[:], ps8[:C, :, :], AF.Exp, bias=b8c_t[:C, 0:1], scale=float(g8)
                )
                pending.append((ex, pi))

            def final_last(h, pi, filler=None):
                # the very last pair: per-tile staggered tail so the two
                # half-chains (exp -> sum -> recip -> mult -> store) pipeline
                # instead of draining serially after the final matmul.
                bs = pi * PW
                ps8 = pp.tile([128, 2, BT], f32, tag="ps", name="ps8")
                for t in range(2):
                    for p in range(KO // 2):
                        nc.tensor.matmul(
                            ps8[:16, t, :],
                            lhsT=w8_t[:, p, :, :],
                            rhs=h[:, p, t, :, :],
                            start=(p == 0),
                            stop=(p == KO // 2 - 1),
                            perf_mode=PM.DoubleRow,
                        )
                    # exp for tile t issues as soon as its chains finish
                    if t == 0:
                        exl = spool.tile([C, 2, BT], bf16, tag="ex", name="exl")
                    nc.scalar.activation(
                        exl[:, t, :], ps8[:C, t, :], AF.Exp,
                        bias=b8c_t[:C, 0:1], scale=float(g8),
                    )
                    if t == 0 and filler is not None:
                        # pending tail's sum-MMs fill the exp-t0 round-trip
                        filler()
                ps_s = pp.tile([128, 2, BT], f32, tag="ps", name="ps_sl")
                rs = spool.tile([C, 2, BT], f32, tag="rs", name="rsl")
                ot = opool.tile([C, 2, BT], f32, tag="ot", name="otl")
                for t in range(2):
                    nc.tensor.matmul(
                        ps_s[:C, t, :], lhsT=ones10[:], rhs=exl[:, t, :],
                        start=True, stop=True,
                    )
                    nc.vector.reciprocal_approx_fast(rs[:, t, :], ps_s[:C, t, :])
                    eng = nc.gpsimd if t == 0 else nc.vector
                    eng.tensor_tensor(ot[:, t, :], exl[:, t, :], rs[:, t, :], ALU.mult)
                    nc.sync.dma_start(
                        y_h.ap()[:, bs + t * BT : bs + (t + 1) * BT], ot[:, t, :]
                    )

            def flush_tail(last=False):
                # class-sum matmul + reciprocal + multiply + store; issued
                # late so PE never waits on the exp round-trip, one tail per
                # layer-1 slot so the shared ps8 ring never stalls PE.
                if not pending:
                    return
                ex, pi = pending.pop(0)
                bs = pi * PW
                ps_s = pp.tile([128, 2, BT], f32, tag="ps", name="ps_s")
                for t in range(2):
                    nc.tensor.matmul(
                        ps_s[:C, t, :], lhsT=ones10[:], rhs=ex[:, t, :],
                        start=True, stop=True,
                    )
                rs = spool.tile([C, 2, BT], f32, tag="rs", name="rs")
                nc.vector.reciprocal_approx_fast(rs[:], ps_s[:C, :, :])
                ot = opool.tile([C, 2, BT], f32, tag="ot", name="ot")
                # multiply on Pool (idle) so only the reciprocal loads DVE;
                # the very last tail overlaps its sibling via DVE.
                eng = nc.vector if last == 2 else nc.gpsimd
                eng.tensor_tensor(ot[:], ex[:], rs[:], ALU.mult)
                nc.sync.dma_start(y_h.ap()[:, bs : bs + PW], ot[:])

            def process_group(prs, is_last=False):
                # FOUR pairs interleaved at layer granularity: every
                # cross-layer dependency (relu/AND chain -> next layer's
                # matmuls) gets three sibling pair-layers (~10us) of slack,
                # so transient ACT/DVE backlogs never stall the PE.
                hs = [p[0] for p in prs]
                n_p = len(prs)
                pend_and = [None]

                def step(l, j):
                    hs[j], afn = hidden_layer(l, hs[j], prs[j][1], parity=j % 2)
                    # previous pair's dropout AND lands AFTER this window's
                    # relus in the DVE queue (its consumer is 3-4 windows
                    # away), so it never delays the critical relu chain.
                    if pend_and[0] is not None:
                        pend_and[0]()
                    pend_and[0] = afn

                for l in range(1, 7):
                    for j in range(n_p):
                        step(l, j)
                        # previous group's softmax tails are all flushed in
                        # the roomy L1 step (6.9us windows): a DVE reciprocal
                        # in a 3.46us hidden window overloads DVE right after
                        # the drop step's AND backlog.
                        if l == 1:
                            flush_tail()
                for j in range(n_p):
                    step(7, j)
                    if is_last and j == n_p - 1:
                        # drain the second-to-last tail while this pair's
                        # L8+exp run, then the very last tail.
                        final_last(
                            hs[j], prs[j][2],
                            filler=lambda: flush_tail(last=2),
                        )
                    else:
                        # L8-j right after L7-j: its tail chain overlaps the
                        # later pairs' L7/L8 instead of draining at the end.
                        final_head(hs[j], prs[j][2])
                        if is_last and j >= 1:
                            flush_tail(last=(1 if j == n_p - 2 else 0))

            process_group(
                [(xt0, mt0, 0), (xt1, mt1, 1), (xt2, mt2, 2), (xt3, mt3, 3)]
            )
            grp = []
            for pi in range(4, npair):
                xg, mg = load_pair(pi)
                grp.append((xg, mg, pi))
            process_group(grp, is_last=True)
            flush_tail(last=2)

    nc.compile()
    return nc


def host_prepare(inputs: dict) -> tuple[dict, dict, float]:
    """Calibrate fp8 scales, quantize weights, compute masks, shard x.

    Returns (shared_inputs, per_core_varying, g8) where per_core_varying maps
    name -> list of 8 per-core arrays.
    """
    import jax

    x = np.asarray(inputs["x"], dtype=np.float32)
    W = {i: np.asarray(inputs[f"W{i}"], dtype=np.float32) for i in range(1, 9)}
    b = {i: np.asarray(inputs[f"b{i}"], dtype=np.float32) for i in range(1, 9)}

    # Dropout masks — bit-exact replication of the reference's PRNG stream.
    cpu = jax.devices("cpu")[0]
    with jax.default_device(cpu):
        dk = jax.random.split(jax.random.key(42), 3)
        keeps = {
            l: np.asarray(
                jax.random.bernoulli(dk[i], KEEP[l], (BATCH, H)), dtype=np.float32
            )
            for i, l in enumerate(DROP_LAYERS)
        }

    # Fold 1/(1-p) into the next layer's weights.
    Wf = dict(W)
    for l in DROP_LAYERS:
        Wf[l + 1] = (W[l + 1] / np.float32(KEEP[l])).astype(np.float32)

    # ---- calibration: fp32 forward on 2048 rows to pick pow2 scales ----
    def pow2(v):
        return np.float32(2.0 ** np.round(np.log2(v)))

    ncal = 2048
    h = x[:ncal]
    s = {0: pow2(8.0 / np.sqrt(np.mean(h**2)))}
    for l in range(1, 8):
        h = np.maximum(h @ Wf[l] + b[l], 0.0)
        if l in DROP_LAYERS:
            h = h * keeps[l][:ncal]
        s[l] = pow2(8.0 / max(np.sqrt(np.mean(h**2)), 1e-6))
    ws8 = pow2(8.0 / np.sqrt(np.mean(Wf[8] ** 2)))
    g8 = float(1.0 / (s[7] * ws8))

    # ---- quantize weights: layer l scale is exactly s_l / s_{l-1} ----
    def pack_dual(Wq, ncol):
        """[pairs*2*128, n_blocks*ncol] -> [128, pairs*n_blocks*2*ncol] with
        each DoubleRow lhsT block [128, 2, ncol] contiguous."""
        K, N = Wq.shape
        pairs, n_blocks = K // 256, N // ncol
        arr = Wq.reshape(pairs, 2, 128, n_blocks, ncol).transpose(2, 0, 3, 1, 4)
        return np.ascontiguousarray(arr.reshape(128, pairs * n_blocks * 2 * ncol))

    def pack_dual_nmajor(Wq, ncol):
        """Like pack_dual but outblock-major: [128, n_blocks*pairs*2*ncol]."""
        K, N = Wq.shape
        pairs, n_blocks = K // 256, N // ncol
        arr = Wq.reshape(pairs, 2, 128, n_blocks, ncol).transpose(2, 3, 0, 1, 4)
        return np.ascontiguousarray(arr.reshape(128, -1))

    W8q = {}
    # w1: 768 features as 3 DoubleRow chunk-pairs; the 16 leftover features
    # (768..783) as a separate [16,128]-per-block tail, replicated into the
    # four 32-row quadrant groups for row-packed K=16 matmuls.
    W1q = (Wf[1] * (s[1] / s[0])).astype(E4)
    W8q[1] = pack_dual_nmajor(np.ascontiguousarray(W1q[:K1_MAIN]), 128)
    w1tail = np.zeros((128, 128), dtype=E4)
    for n in range(KO):
        w1tail[32 * n : 32 * n + K1_TAIL, :] = W1q[K1_MAIN:, 128 * n : 128 * (n + 1)]
    for l in range(2, 8):
        W8q[l] = pack_dual((Wf[l] * (s[l] / s[l - 1])).astype(E4), 128)
    W8p = np.zeros((H, 16), dtype=np.float32)
    W8p[:, :C] = Wf[8] * ws8
    W8q[8] = pack_dual(W8p.astype(E4), 16)

    # biases: s_l * b_l, packed [128, 4] per layer
    bias17 = np.empty((128, 28), dtype=np.float32)
    for l in range(1, 8):
        bias17[:, (l - 1) * 4 : l * 4] = (s[l] * b[l]).reshape(4, 128).T
    b8c = np.zeros((128, 1), dtype=np.float32)
    b8c[:C, 0] = b[8]

    # x: quantize, transpose; 768 features DR-packed + 16-feature tail
    # replicated across the four row-quadrant partition groups
    xTq = (x.T * s[0]).astype(E4)
    xTp = np.ascontiguousarray(xTq[:K1_MAIN])
    xtail = np.zeros((128, BATCH), dtype=E4)
    for q in range(KO):
        xtail[32 * q : 32 * q + K1_TAIL, :] = xTq[K1_MAIN:]

    def pack_act(a):
        """[F, B_CORE] feature-major -> [128, npair*F/128*2*BT] in the SBUF
        pair layout [p, pair, ko_pair, tile, slot, BT]."""
        F, Bc = a.shape
        v = a.reshape(F // 256, 2, 128, Bc // PW, 2, BT)  # [pr, sl, p, pair, t, b]
        v = v.transpose(2, 3, 0, 4, 1, 5)                 # [p, pair, pr, t, sl, b]
        return np.ascontiguousarray(v.reshape(128, -1))

    def pack_x(a):
        """Tile-major variant for x: [p, pair, tile, ko_pair, slot, BT] so
        each batch tile's half is one contiguous DMA."""
        F, Bc = a.shape
        v = a.reshape(F // 256, 2, 128, Bc // PW, 2, BT)  # [pr, sl, p, pair, t, b]
        v = v.transpose(2, 3, 4, 0, 1, 5)                 # [p, pair, t, pr, sl, b]
        return np.ascontiguousarray(v.reshape(128, -1))

    shared = {
        "w1": W8q[1],
        "w1t": w1tail,
        "w8": W8q[8],
        "bias17": bias17,
        "b8c": b8c,
    }
    for l in range(2, 8):
        shared[f"w{l}"] = W8q[l]

    per_core = {"xT": [], "xtl": [], "m2": [], "m4": [], "m6": []}
    mT = {
        l: np.where(keeps[l].T != 0, 255, 0).astype(np.uint8) for l in DROP_LAYERS
    }
    for c in range(N_CORES):
        sl = slice(c * B_CORE, (c + 1) * B_CORE)
        per_core["xT"].append(pack_x(xTp[:, sl]))
        # xtail pair layout: [128, pair, tile, BT]
        xt = xtail[:, sl].reshape(128, B_CORE // PW, 2, BT)
        per_core["xtl"].append(np.ascontiguousarray(xt.reshape(128, -1)))
        for l in DROP_LAYERS:
            per_core[f"m{l}"].append(pack_act(mT[l][:, sl]).view(np.uint32))
    return shared, per_core, g8


def run_hw(inputs: dict, trace: bool = False):
    from concourse import bass_utils

    shared, per_core, g8 = host_prepare(inputs)
    nc = build_bass(B_CORE, g8)
    in_maps = [
        {**shared, **{k: v[c] for k, v in per_core.items()}} for c in range(N_CORES)
    ]
    res = bass_utils.run_bass_kernel_spmd(
        nc, in_maps, core_ids=list(range(N_CORES)), trace=trace
    )
    out = np.concatenate([np.ascontiguousarray(r["yT"].T) for r in res.results], axis=0)
    return out.astype(np.float32), res


def kernel(**inputs) -> np.ndarray:
    return run_hw(inputs, trace=False)[0]



# revision 66
# speedup vs baseline: 1.0151x; 1.0074x over previous
"""Trainium2 Bass kernel for an 8-layer dense MLP (784->512x6->10) + softmax.

Strategy (hardcoded for batch=65536, 8 NeuronCores, pure data parallel):
  - Each core handles 8192 rows of the batch; weights replicated.
  - All matmuls run in fp8-e4m3 with MatmulPerfMode.DoubleRow (256-feature
    contraction per instruction; one 512-px matmul issues every ~216ns =
    the fp8 peak).  PE is the bottleneck: everything else is scheduled to
    keep its 216ns cadence unbroken.
  - Layer 1 contracts 768 of the 784 input features with 3 DoubleRow chunks
    and handles the 16 leftover features as K=16 normal-mode matmuls packed
    two-concurrent onto 32-row array quadrants via tile_position - saving
    most of the pad-to-1024 waste.
  - Quantization scales: per-layer power-of-2 activation scales s_l picked by
    a 2048-row fp32 calibration forward on host; weight scale for layer l is
    exactly s_l/s_{l-1}, so PSUM already carries s_l * preactivation and the
    PSUM->fp8 step is a single fused  q8(max(psum + s_l*b_l, 0))  on either
    ACT (activation, bias=) or DVE (tensor_scalar add,max).
  - FOUR pairs (of two BT=512 batch tiles) are in flight, interleaved at
    layer granularity: every relu/AND -> next-layer dependency gets three
    sibling pair-layer windows (~10us) of slack.  Drop layers alternate the
    relu split (3 ACT/1 DVE vs 2/2+1) per pair parity so neither post-op
    engine runs saturated for four consecutive windows.
  - Dropout masks (jax threefry, key 42) are bit-exactly precomputed on host
    and shipped as u32 {0x00,0xFF}-byte words; applied as ONE whole-layer
    bitwise-AND on DVE whose emission is deferred past the next pair's
    relus (in-order DVE FIFO: an early AND delays the critical relus).
    1/(1-p) is folded into the next layer's weights on host.
  - Softmax: exp on ACT -> bf16 (scale=g8 dequant, bias=b8), replicated
    class-sum via a [10,10] all-ones matmul on PE, reciprocal_approx_fast on
    DVE + multiply on Pool.  Tails are deferred and flushed in the next
    group's roomy L1 windows; the final group drains them staggered between
    the last pairs' L7/L8 so the kernel tail stays short.
  - All PSUM flows through one 4-slot ring of [128,2,BT] two-bank pair
    tiles (8 banks total).  Startup: x + hidden weights + masks stream on
    the Pool software-DGE queue (~250 GB/s measured; HW-DGE queues only
    ~78 GB/s) in first-use order; only the small w1/bias tiles ride the
    HW-DGE queues, and nothing with a semaphore wait is ever placed on a
    compute engine's queue (a gated DMA trigger head-of-line-blocks it).
"""

import numpy as np
import ml_dtypes

E4 = ml_dtypes.float8_e4m3

BATCH = 65536
D_IN = 784
K1_MAIN = 768              # input features contracted via 3 DoubleRow pairs
K1P = K1_MAIN // 256       # = 3 DR chunk-pairs for layer 1
K1_TAIL = D_IN - K1_MAIN   # = 16 leftover features, row-quadrant packed
H = 512
KO = H // 128              # 4 feature chunks for hidden layers
C = 10
N_CORES = 8
B_CORE = BATCH // N_CORES  # 8192
BT = 512                   # batch tile (matmul moving free dim)
PW = 2 * BT                # pair width

DROP_LAYERS = (2, 4, 6)    # dropout applied to these layers' outputs
KEEP = {2: 0.8, 4: 0.7, 6: 0.5}


def build_bass(b_core: int, g8: float):
    """Build the Bass module for one core processing b_core batch rows."""
    import concourse.mybir as mybir
    import concourse.tile as tile
    from concourse import bacc

    f32 = mybir.dt.float32
    f8 = mybir.dt.float8e4
    bf16 = mybir.dt.bfloat16
    AF = mybir.ActivationFunctionType
    ALU = mybir.AluOpType
    PM = mybir.MatmulPerfMode

    npair = b_core // PW

    nc = bacc.Bacc("TRN2", target_bir_lowering=False, debug=False)

    # Weights are pre-packed on host so every DoubleRow lhsT block
    # [128, 2, ncol] is contiguous in SBUF (s3_lw_dual_fp8_restrictions):
    # layer l image is [128, pairs, KO, 2, 128] flattened to 2D.
    # x and masks are pre-packed into the SBUF pair layout
    # [p, pair, ko_pair, tile, slot, BT] flattened to 2D so each pair loads
    # with one fully-contiguous DMA.
    xT = nc.dram_tensor("xT", [128, npair * K1P * 2 * 2 * BT], f8, kind="ExternalInput")
    xtl_h = nc.dram_tensor("xtl", [128, npair * 2 * BT], f8, kind="ExternalInput")
    w_h = {1: nc.dram_tensor("w1", [128, K1P * KO * 256], f8, kind="ExternalInput")}
    w1t_h = nc.dram_tensor("w1t", [128, 128], f8, kind="ExternalInput")
    for l in range(2, 8):
        w_h[l] = nc.dram_tensor(f"w{l}", [128, (KO // 2) * KO * 256], f8, kind="ExternalInput")
    w8_h = nc.dram_tensor("w8", [128, (KO // 2) * 2 * 16], f8, kind="ExternalInput")
    bias17_h = nc.dram_tensor("bias17", [128, 28], f32, kind="ExternalInput")
    b8c_h = nc.dram_tensor("b8c", [128, 1], f32, kind="ExternalInput")
    u32 = mybir.dt.uint32
    # masks ship as uint32 {0x00.., 0xFF..} bytes covering four fp8 lanes
    # each: the dropout multiply is a bitwise AND on u32 views (DVE only —
    # Pool has no bitwise ops and DVE bitwise requires 32-bit).
    m_h = {
        l: nc.dram_tensor(f"m{l}", [128, npair * KO * BT // 2], u32, kind="ExternalInput")
        for l in DROP_LAYERS
    }
    y_h = nc.dram_tensor("yT", [C, b_core], f32, kind="ExternalOutput")

    # Per-layer relu/quantize engine map (A=ACT, D=DVE), one char per output
    # block n=0..3.  Drop layers alternate the 3:1 split per pair parity so
    # neither engine runs >99% for four consecutive windows (the mask-AND
    # rides DVE); other layers split evenly.  Block 0 is always ACT-first:
    # the next pair-layer's first chain WAR-waits on it via the PSUM ring.
    RELU_ENG = {
        1: "ADAD", 2: ("AADA", "ADDA"), 3: "ADAD", 4: ("AADA", "ADDA"),
        5: "ADAD", 6: ("AADA", "ADDA"), 7: "ADAD",
    }

    with tile.TileContext(nc) as tc:
        with (
            tc.tile_pool(name="wpool", bufs=1) as wpool,
            tc.tile_pool(name="xpool", bufs=5) as xpool,
            tc.tile_pool(name="hpool", bufs=8) as hpool,
            tc.tile_pool(name="mpool", bufs=5) as mpool,
            tc.tile_pool(name="spool", bufs=3) as spool,
            tc.tile_pool(name="opool", bufs=3) as opool,
            tc.tile_pool(name="psum", bufs=4, space="PSUM") as pp,
        ):
            # activation/x/mask pair layout: [128, ko_pair, tile, slot, BT]
            # so a DoubleRow rhs block [128, 2, BT] is contiguous per tile;
            # x and masks arrive pre-packed in this layout (one contiguous
            # DMA per pair).
            XF = K1P * 2 * 2 * BT  # x free elems per partition per pair
            XTF = 2 * BT       # xtail elems per partition per pair

            MF = KO * BT // 2  # mask u32 elems per partition per pair

            gate = {"inst": None}
            cur_xtl = {"tile": None}

            def load_mask(l, pi, queue=None, gate_on=None):
                mtl = mpool.tile([128, KO // 2, 2, 2, BT // 4], u32, tag=f"m{l}", name=f"m{l}_t")
                mi = (queue or nc.gpsimd).dma_start(
                    mtl[:], m_h[l].ap()[:, pi * MF : (pi + 1) * MF]
                )
                if gate_on is not None:
                    tile.add_dep_helper(mi.ins, gate_on, sync=True)
                if gate["inst"] is not None:
                    tile.add_dep_helper(mi.ins, gate["inst"], sync=True)
                return mtl

            def load_x(pi, queue):
                # x is packed tile-major; the two batch tiles of a pair are
                # SEPARATE SBUF tiles (dep tracking is per-tile, so an L1
                # chain starts as soon as its own tile's 512KB half lands).
                # Returns per-tile rhs getters (p -> AP) so pair 0 can use a
                # different tiling without changing hidden_layer.
                xts = []
                for t, tag in ((0, "xta"), (1, "xtb")):
                    xt = xpool.tile([128, K1P, 2, BT], f8, tag=tag, name=tag)
                    di = queue.dma_start(
                        xt[:],
                        xT.ap()[:, pi * XF + t * (XF // 2) : pi * XF + (t + 1) * (XF // 2)],
                    )
                    if gate["inst"] is not None:
                        tile.add_dep_helper(di.ins, gate["inst"], sync=True)
                    xts.append(xt)
                if pi % 2 == 0:
                    # one xtail image per TWO pairs (256KB): small enough
                    # not to delay the next pair's x on the critical queue
                    xtl = xpool.tile([128, 2, 2, BT], f8, tag="xtl", name="xtl")
                    di = queue.dma_start(
                        xtl[:], xtl_h.ap()[:, pi * XTF : (pi + 2) * XTF]
                    )
                    if gate["inst"] is not None:
                        tile.add_dep_helper(di.ins, gate["inst"], sync=True)
                    cur_xtl["tile"] = xtl
                getters = tuple((lambda p, x=x: x[:, p, :, :]) for x in xts)
                return (getters, cur_xtl["tile"], pi % 2), di

            def load_pair(pi):
                # x on the SP HW-DGE queue; weights + masks on the Pool
                # (software DGE) queue - parallel HBM streams, ACT untouched.
                xg, _ = load_x(pi, nc.sync)
                return xg, {l: load_mask(l, pi) for l in DROP_LAYERS}

            # Startup: x pairs 0/1 stream on the Pool software-DGE queue
            # (~300 GB/s) ahead of the hidden weights + first masks; w1 is
            # packed outblock-major as FOUR SEPARATE TILES on the SP queue so
            # the first L1 chain depends on just its own 128KB chunk (dep
            # tracking is per-tile, so chunks into one tile don't help).
            # Warm the PE clock during the fixed ~13us startup window (engine
            # preamble + DMA trigger path) so the first real matmuls run at
            # full p-state instead of ramping through L1.
            warm_w = wpool.tile([128, 2, 128], f8, tag="warmw")
            warm_x = wpool.tile([128, 2, BT], f8, tag="warmx")
            nc.vector.memset(warm_w[:], 0)
            nc.vector.memset(warm_x[:], 0)
            warm_ps = pp.tile([128, 2, BT], f32, tag="ps", name="warm_ps")
            for _ in range(6):
                nc.tensor.matmul(
                    warm_ps[:, 0, :], lhsT=warm_w[:], rhs=warm_x[:],
                    start=True, stop=True, perf_mode=PM.DoubleRow,
                )

            # Startup: x pairs 0/1 + hidden weights stream on the Pool
            # software-DGE queue (~250 GB/s measured; the HW-DGE queues only
            # do ~78 GB/s); w1 blocks + bias go on the slow-but-parallel SP /
            # ACT HW-DGE queues as four separate tiles so the first L1 chain
            # depends on just its own 128KB block.
            W1F = K1P * 256
            w1n = [
                wpool.tile([128, K1P, 2, 128], f8, tag=f"w1n{n}", name=f"w1n{n}")
                for n in range(KO)
            ]
            bias17_t = wpool.tile([128, 28], f32, tag="bias17")
            nc.sync.dma_start(bias17_t[:], bias17_h.ap())
            w1t_t = wpool.tile([128, 128], f8, tag="w1t")
            nc.sync.dma_start(w1t_t[:], w1t_h.ap())
            nc.sync.dma_start(w1n[0][:], w_h[1].ap()[:, 0:W1F])
            nc.scalar.dma_start(w1n[1][:], w_h[1].ap()[:, W1F : 2 * W1F])
            nc.sync.dma_start(w1n[2][:], w_h[1].ap()[:, 2 * W1F : 3 * W1F])
            nc.scalar.dma_start(w1n[3][:], w_h[1].ap()[:, 3 * W1F : 4 * W1F])
            # Startup pairs 0-3: x + hidden weights + all 12 mask images on
            # the fast Pool SWDGE queue, ordered by first use (4-pair
            # interleave gives the masks until ~42us).  NOTHING with a
            # semaphore wait goes on the ACT/scalar queue - a gated DMA
            # trigger head-of-line-blocks the engine's FIFO and stalls its
            # relus (measured 17us ACT stall).
            # Pair 0 is the DMA critical path: split tile-a into a 256KB
            # chunk-pair half and a 128KB remainder so the very first L1
            # chain starts ~1us earlier (p0/p1 consume the first half, p2
            # lands ~0.5us later, just in time for the 3rd chain slot).
            x0a0 = wpool.tile([128, 2, 2, BT], f8, tag="x0a0")
            nc.gpsimd.dma_start(x0a0[:], xT.ap()[:, 0 : 4 * BT])
            x0a1 = wpool.tile([128, 1, 2, BT], f8, tag="x0a1")
            nc.gpsimd.dma_start(x0a1[:], xT.ap()[:, 4 * BT : 6 * BT])
            x0b = wpool.tile([128, K1P, 2, BT], f8, tag="x0b")
            nc.gpsimd.dma_start(x0b[:], xT.ap()[:, XF // 2 : XF])
            xtl0 = xpool.tile([128, 2, 2, BT], f8, tag="xtl", name="xtl0")
            nc.gpsimd.dma_start(xtl0[:], xtl_h.ap()[:, 0 : 2 * XTF])
            cur_xtl["tile"] = xtl0
            xt0 = (
                (
                    lambda p: x0a0[:, p, :, :] if p < 2 else x0a1[:, 0, :, :],
                    lambda p: x0b[:, p, :, :],
                ),
                xtl0,
                0,
            )
            w_t = {}
            for l in range(2, 8):
                w_t[l] = wpool.tile([128, KO // 2, KO, 2, 128], f8, tag=f"w{l}", name=f"w{l}_t")
            nc.gpsimd.dma_start(w_t[2][:], w_h[2].ap())
            xt1, _ = load_x(1, nc.gpsimd)
            nc.gpsimd.dma_start(w_t[3][:], w_h[3].ap())
            xt2, _ = load_x(2, nc.gpsimd)
            nc.gpsimd.dma_start(w_t[4][:], w_h[4].ap())
            nc.gpsimd.dma_start(w_t[5][:], w_h[5].ap())
            xt3, _ = load_x(3, nc.gpsimd)
            nc.gpsimd.dma_start(w_t[6][:], w_h[6].ap())
            w7_dma = nc.gpsimd.dma_start(w_t[7][:], w_h[7].ap())
            w8_t = wpool.tile([128, KO // 2, 2, 16], f8, tag="w8")
            nc.gpsimd.dma_start(w8_t[:], w8_h.ap())
            b8c_t = wpool.tile([128, 1], f32, tag="b8c")
            nc.gpsimd.dma_start(b8c_t[:], b8c_h.ap())
            mt0, mt1, mt2, mt3 = {}, {}, {}, {}
            for l in DROP_LAYERS:
                mt0[l] = load_mask(l, 0)
                mt1[l] = load_mask(l, 1)
                mt2[l] = load_mask(l, 2)
                mt3[l] = load_mask(l, 3)
            ones10 = wpool.tile([C, C], bf16, tag="ones10")
            nc.vector.memset(ones10[:], 1.0)
            gate["inst"] = w7_dma.ins

            def relu_pair(dst, ps, bias_ap, eng):
                # q8(max(psum + s_l*b_l, 0)) for both tiles, PSUM -> fp8
                if eng == "A":
                    nc.scalar.activation(dst, ps, AF.Relu, bias=bias_ap)
                else:
                    nc.vector.tensor_scalar(dst, ps, bias_ap, 0.0, ALU.add, ALU.max)

            def layer1(src, eng):
                # L1 contracts 768 features via 3 DoubleRow chunks per block
                # plus a K=16 leftover matmul per block.  The four leftover
                # matmuls of a kp-group run CONCURRENTLY on distinct 32-row
                # quadrants (tile_position row packing): ~2 slot times
                # instead of 4, saving 4 x 216ns per pair vs padding to 1024.
                getters, xtl, sub = src
                hn = hpool.tile([128, KO // 2, 2, 2, BT], f8, tag="h", name="h")
                for kp in range(2):
                    ns = (2 * kp, 2 * kp + 1)
                    pss = {n: pp.tile([128, 2, BT], f32, tag="ps", name="ps") for n in ns}
                    for n in ns:
                        for t in range(2):
                            for p in range(K1P):
                                nc.tensor.matmul(
                                    pss[n][:, t, :],
                                    lhsT=w1n[n][:, p, :, :],
                                    rhs=getters[t](p),
                                    start=(p == 0),
                                    stop=False,
                                    perf_mode=PM.DoubleRow,
                                    skip_group_check=True,
                                )
                    # K=16 leftover matmuls close both blocks' groups as a
                    # row-quadrant-packed burst (two concurrent per round).
                    for t in range(2):
                        for n in ns:
                            nc.tensor.matmul(
                                pss[n][:, t, :],
                                lhsT=w1t_t[32 * n : 32 * n + K1_TAIL, :],
                                rhs=xtl[32 * n : 32 * n + K1_TAIL, sub, t, :],
                                start=False,
                                stop=True,
                                tile_position=(32 * n, 0),
                                skip_group_check=True,
                            )
                    for n in ns:
                        relu_pair(
                            hn[:, kp, :, n % 2, :],
                            pss[n][:],
                            bias17_t[:, n : n + 1],
                            eng[n],
                        )
                return hn

            def hidden_layer(l, src, mt, parity=0):
                pairs_in = K1P if l == 1 else KO // 2
                eng = RELU_ENG[l]
                if isinstance(eng, tuple):
                    eng = eng[parity]
                if l == 1:
                    return layer1(src, eng), None
                hn = hpool.tile([128, KO // 2, 2, 2, BT], f8, tag="h", name="h")
                # Drop layers process block 2 (the DVE relu) LAST so the
                # single whole-layer AND directly follows it on DVE and its
                # PSUM-ring slot is the last one the next pair-layer
                # overwrites.
                order = (0, 1, 3, 2) if l in DROP_LAYERS else range(KO)
                for n in order:
                    ps = pp.tile([128, 2, BT], f32, tag="ps", name="ps")
                    for t in range(2):
                        for p in range(pairs_in):
                            if l == 1:
                                lhsT = w1n[n][:, p, :, :]
                                rhs = src[t](p)  # x: per-tile rhs getters
                            else:
                                lhsT = w_t[l][:, p, n, :, :]
                                rhs = src[:, p, t, :, :]
                            nc.tensor.matmul(
                                ps[:, t, :],
                                lhsT=lhsT,
                                rhs=rhs,
                                start=(p == 0),
                                stop=(p == pairs_in - 1),
                                perf_mode=PM.DoubleRow,
                            )
                    relu_pair(
                        hn[:, n // 2, :, n % 2, :],
                        ps[:],
                        bias17_t[:, (l - 1) * 4 + n : (l - 1) * 4 + n + 1],
                        eng[n],
                    )
                if l in DROP_LAYERS:
                    # The AND's emission is DEFERRED by the caller to after
                    # the NEXT pair's relus: a whole-layer AND sitting ahead
                    # of the following window's relus in the in-order DVE
                    # FIFO delays them and stalls the PE via the PSUM ring.
                    def and_fn(hn=hn, l=l, mt=mt):
                        d32 = hn[:].bitcast(u32)
                        nc.vector.tensor_tensor(
                            d32, d32, mt[l][:, :, :, :, :], ALU.bitwise_and
                        )
                    return hn, and_fn
                return hn, None

            pending = []

            def final_head(h, pi):
                # layer 8 (512->10 padded 16) for both tiles + exp -> bf16.
                ps8 = pp.tile([128, 2, BT], f32, tag="ps", name="ps8")
                for t in range(2):
                    for p in range(KO // 2):
                        nc.tensor.matmul(
                            ps8[:16, t, :],
                            lhsT=w8_t[:, p, :, :],
                            rhs=h[:, p, t, :, :],
                            start=(p == 0),
                            stop=(p == KO // 2 - 1),
                            perf_mode=PM.DoubleRow,
                        )
                ex = spool.tile([C, 2, BT], bf16, tag="ex", name="ex")
                nc.scalar.activation(
                    ex[:], ps8[:C, :, :], AF.Exp, bias=b8c_t[:C, 0:1], scale=float(g8)
                )
                pending.append((ex, pi))

            def final_last(h, pi, filler=None):
                if filler is not None:
                    # half the pending tail fills the L7-relu wait
                    filler[0]()
                # the very last pair: per-tile staggered tail so the two
                # half-chains (exp -> sum -> recip -> mult -> store) pipeline
                # instead of draining serially after the final matmul.
                bs = pi * PW
                ps8 = pp.tile([128, 2, BT], f32, tag="ps", name="ps8")
                for t in range(2):
                    for p in range(KO // 2):
                        nc.tensor.matmul(
                            ps8[:16, t, :],
                            lhsT=w8_t[:, p, :, :],
                            rhs=h[:, p, t, :, :],
                            start=(p == 0),
                            stop=(p == KO // 2 - 1),
                            perf_mode=PM.DoubleRow,
                        )
                    # exp for tile t issues as soon as its chains finish
                    if t == 0:
                        exl = spool.tile([C, 2, BT], bf16, tag="ex", name="exl")
                    nc.scalar.activation(
                        exl[:, t, :], ps8[:C, t, :], AF.Exp,
                        bias=b8c_t[:C, 0:1], scale=float(g8),
                    )
                    if t == 0 and filler is not None:
                        # the rest fills the exp-t0 round-trip
                        filler[1]()
                ps_s = pp.tile([128, 2, BT], f32, tag="ps", name="ps_sl")
                rs = spool.tile([C, 2, BT], f32, tag="rs", name="rsl")
                ot = opool.tile([C, 2, BT], f32, tag="ot", name="otl")
                for t in range(2):
                    nc.tensor.matmul(
                        ps_s[:C, t, :], lhsT=ones10[:], rhs=exl[:, t, :],
                        start=True, stop=True,
                    )
                    nc.vector.reciprocal_approx_fast(rs[:, t, :], ps_s[:C, t, :])
                    eng = nc.gpsimd if t == 0 else nc.vector
                    eng.tensor_tensor(ot[:, t, :], exl[:, t, :], rs[:, t, :], ALU.mult)
                    nc.sync.dma_start(
                        y_h.ap()[:, bs + t * BT : bs + (t + 1) * BT], ot[:, t, :]
                    )

            def flush_tail(last=False):
                # class-sum matmul + reciprocal + multiply + store; issued
                # late so PE never waits on the exp round-trip, one tail per
                # layer-1 slot so the shared ps8 ring never stalls PE.
                if not pending:
                    return
                ex, pi = pending.pop(0)
                bs = pi * PW
                ps_s = pp.tile([128, 2, BT], f32, tag="ps", name="ps_s")
                for t in range(2):
                    nc.tensor.matmul(
                        ps_s[:C, t, :], lhsT=ones10[:], rhs=ex[:, t, :],
                        start=True, stop=True,
                    )
                rs = spool.tile([C, 2, BT], f32, tag="rs", name="rs")
                nc.vector.reciprocal_approx_fast(rs[:], ps_s[:C, :, :])
                ot = opool.tile([C, 2, BT], f32, tag="ot", name="ot")
                # multiply on Pool (idle) so only the reciprocal loads DVE;
                # the very last tail overlaps its sibling via DVE.
                eng = nc.vector if last == 2 else nc.gpsimd
                eng.tensor_tensor(ot[:], ex[:], rs[:], ALU.mult)
                nc.sync.dma_start(y_h.ap()[:, bs : bs + PW], ot[:])

            def process_group(prs, is_last=False):
                # FOUR pairs interleaved at layer granularity: every
                # cross-layer dependency (relu/AND chain -> next layer's
                # matmuls) gets three sibling pair-layers (~10us) of slack,
                # so transient ACT/DVE backlogs never stall the PE.
                hs = [p[0] for p in prs]
                n_p = len(prs)
                pend_and = [None]

                def step(l, j):
                    hs[j], afn = hidden_layer(l, hs[j], prs[j][1], parity=j % 2)
                    # previous pair's dropout AND lands AFTER this window's
                    # relus in the DVE queue (its consumer is 3-4 windows
                    # away), so it never delays the critical relu chain.
                    if pend_and[0] is not None:
                        pend_and[0]()
                    pend_and[0] = afn

                for l in range(1, 7):
                    for j in range(n_p):
                        step(l, j)
                        # previous group's softmax tails are all flushed in
                        # the roomy L1 step (6.9us windows): a DVE reciprocal
                        # in a 3.46us hidden window overloads DVE right after
                        # the drop step's AND backlog.
                        if l == 1:
                            flush_tail()
                for j in range(n_p):
                    step(7, j)
                    if is_last and j == n_p - 1:
                        # drain the second-to-last tail while this pair's
                        # L8+exp run, then the very last tail.
                        fstate = {}

                        def fill0():
                            ex, fpi = pending.pop(0)
                            ps_f = pp.tile([128, 2, BT], f32, tag="ps", name="ps_f")
                            nc.tensor.matmul(
                                ps_f[:C, 0, :], lhsT=ones10[:], rhs=ex[:, 0, :],
                                start=True, stop=True,
                            )
                            fstate["x"] = (ex, fpi, ps_f)

                        def fill1():
                            ex, fpi, ps_f = fstate["x"]
                            nc.tensor.matmul(
                                ps_f[:C, 1, :], lhsT=ones10[:], rhs=ex[:, 1, :],
                                start=True, stop=True,
                            )
                            rs = spool.tile([C, 2, BT], f32, tag="rs", name="rs_f")
                            ot = opool.tile([C, 2, BT], f32, tag="ot", name="ot_f")
                            nc.vector.reciprocal_approx_fast(rs[:], ps_f[:C, :, :])
                            nc.vector.tensor_tensor(ot[:], ex[:], rs[:], ALU.mult)
                            nc.sync.dma_start(
                                y_h.ap()[:, fpi * PW : fpi * PW + PW], ot[:]
                            )

                        final_last(hs[j], prs[j][2], filler=(fill0, fill1))
                    else:
                        # L8-j right after L7-j: its tail chain overlaps the
                        # later pairs' L7/L8 instead of draining at the end.
                        final_head(hs[j], prs[j][2])
                        if is_last and j >= 1:
                            flush_tail(last=(1 if j == n_p - 2 else 0))

            process_group(
                [(xt0, mt0, 0), (xt1, mt1, 1), (xt2, mt2, 2), (xt3, mt3, 3)]
            )
            grp = []
            for pi in range(4, npair):
                xg, mg = load_pair(pi)
                grp.append((xg, mg, pi))
            process_group(grp, is_last=True)
            flush_tail(last=2)

    nc.compile()
    return nc


def host_prepare(inputs: dict) -> tuple[dict, dict, float]:
    """Calibrate fp8 scales, quantize weights, compute masks, shard x.

    Returns (shared_inputs, per_core_varying, g8) where per_core_varying maps
    name -> list of 8 per-core arrays.
    """
    import jax

    x = np.asarray(inputs["x"], dtype=np.float32)
    W = {i: np.asarray(inputs[f"W{i}"], dtype=np.float32) for i in range(1, 9)}
    b = {i: np.asarray(inputs[f"b{i}"], dtype=np.float32) for i in range(1, 9)}

    # Dropout masks — bit-exact replication of the reference's PRNG stream.
    cpu = jax.devices("cpu")[0]
    with jax.default_device(cpu):
        dk = jax.random.split(jax.random.key(42), 3)
        keeps = {
            l: np.asarray(
                jax.random.bernoulli(dk[i], KEEP[l], (BATCH, H)), dtype=np.float32
            )
            for i, l in enumerate(DROP_LAYERS)
        }

    # Fold 1/(1-p) into the next layer's weights.
    Wf = dict(W)
    for l in DROP_LAYERS:
        Wf[l + 1] = (W[l + 1] / np.float32(KEEP[l])).astype(np.float32)

    # ---- calibration: fp32 forward on 2048 rows to pick pow2 scales ----
    def pow2(v):
        return np.float32(2.0 ** np.round(np.log2(v)))

    ncal = 2048
    h = x[:ncal]
    s = {0: pow2(8.0 / np.sqrt(np.mean(h**2)))}
    for l in range(1, 8):
        h = np.maximum(h @ Wf[l] + b[l], 0.0)
        if l in DROP_LAYERS:
            h = h * keeps[l][:ncal]
        s[l] = pow2(8.0 / max(np.sqrt(np.mean(h**2)), 1e-6))
    ws8 = pow2(8.0 / np.sqrt(np.mean(Wf[8] ** 2)))
    g8 = float(1.0 / (s[7] * ws8))

    # ---- quantize weights: layer l scale is exactly s_l / s_{l-1} ----
    def pack_dual(Wq, ncol):
        """[pairs*2*128, n_blocks*ncol] -> [128, pairs*n_blocks*2*ncol] with
        each DoubleRow lhsT block [128, 2, ncol] contiguous."""
        K, N = Wq.shape
        pairs, n_blocks = K // 256, N // ncol
        arr = Wq.reshape(pairs, 2, 128, n_blocks, ncol).transpose(2, 0, 3, 1, 4)
        return np.ascontiguousarray(arr.reshape(128, pairs * n_blocks * 2 * ncol))

    def pack_dual_nmajor(Wq, ncol):
        """Like pack_dual but outblock-major: [128, n_blocks*pairs*2*ncol]."""
        K, N = Wq.shape
        pairs, n_blocks = K // 256, N // ncol
        arr = Wq.reshape(pairs, 2, 128, n_blocks, ncol).transpose(2, 3, 0, 1, 4)
        return np.ascontiguousarray(arr.reshape(128, -1))

    W8q = {}
    # w1: 768 features as 3 DoubleRow chunk-pairs; the 16 leftover features
    # (768..783) as a separate [16,128]-per-block tail, replicated into the
    # four 32-row quadrant groups for row-packed K=16 matmuls.
    W1q = (Wf[1] * (s[1] / s[0])).astype(E4)
    W8q[1] = pack_dual_nmajor(np.ascontiguousarray(W1q[:K1_MAIN]), 128)
    w1tail = np.zeros((128, 128), dtype=E4)
    for n in range(KO):
        w1tail[32 * n : 32 * n + K1_TAIL, :] = W1q[K1_MAIN:, 128 * n : 128 * (n + 1)]
    for l in range(2, 8):
        W8q[l] = pack_dual((Wf[l] * (s[l] / s[l - 1])).astype(E4), 128)
    W8p = np.zeros((H, 16), dtype=np.float32)
    W8p[:, :C] = Wf[8] * ws8
    W8q[8] = pack_dual(W8p.astype(E4), 16)

    # biases: s_l * b_l, packed [128, 4] per layer
    bias17 = np.empty((128, 28), dtype=np.float32)
    for l in range(1, 8):
        bias17[:, (l - 1) * 4 : l * 4] = (s[l] * b[l]).reshape(4, 128).T
    b8c = np.zeros((128, 1), dtype=np.float32)
    b8c[:C, 0] = b[8]

    # x: quantize, transpose; 768 features DR-packed + 16-feature tail
    # replicated across the four row-quadrant partition groups
    xTq = (x.T * s[0]).astype(E4)
    xTp = np.ascontiguousarray(xTq[:K1_MAIN])
    xtail = np.zeros((128, BATCH), dtype=E4)
    for q in range(KO):
        xtail[32 * q : 32 * q + K1_TAIL, :] = xTq[K1_MAIN:]

    def pack_act(a):
        """[F, B_CORE] feature-major -> [128, npair*F/128*2*BT] in the SBUF
        pair layout [p, pair, ko_pair, tile, slot, BT]."""
        F, Bc = a.shape
        v = a.reshape(F // 256, 2, 128, Bc // PW, 2, BT)  # [pr, sl, p, pair, t, b]
        v = v.transpose(2, 3, 0, 4, 1, 5)                 # [p, pair, pr, t, sl, b]
        return np.ascontiguousarray(v.reshape(128, -1))

    def pack_x(a):
        """Tile-major variant for x: [p, pair, tile, ko_pair, slot, BT] so
        each batch tile's half is one contiguous DMA."""
        F, Bc = a.shape
        v = a.reshape(F // 256, 2, 128, Bc // PW, 2, BT)  # [pr, sl, p, pair, t, b]
        v = v.transpose(2, 3, 4, 0, 1, 5)                 # [p, pair, t, pr, sl, b]
        return np.ascontiguousarray(v.reshape(128, -1))

    shared = {
        "w1": W8q[1],
        "w1t": w1tail,
        "w8": W8q[8],
        "bias17": bias17,
        "b8c": b8c,
    }
    for l in range(2, 8):
        shared[f"w{l}"] = W8q[l]

    per_core = {"xT": [], "xtl": [], "m2": [], "m4": [], "m6": []}
    mT = {
        l: np.where(keeps[l].T != 0, 255, 0).astype(np.uint8) for l in DROP_LAYERS
    }
    for c in range(N_CORES):
        sl = slice(c * B_CORE, (c + 1) * B_CORE)
        per_core["xT"].append(pack_x(xTp[:, sl]))
        # xtail pair layout: [128, pair, tile, BT]
        xt = xtail[:, sl].reshape(128, B_CORE // PW, 2, BT)
        per_core["xtl"].append(np.ascontiguousarray(xt.reshape(128, -1)))
        for l in DROP_LAYERS:
            per_core[f"m{l}"].append(pack_act(mT[l][:, sl]).view(np.uint32))
    return shared, per_core, g8


def run_hw(inputs: dict, trace: bool = False):
    from concourse import bass_utils

    shared, per_core, g8 = host_prepare(inputs)
    nc = build_bass(B_CORE, g8)
    in_maps = [
        {**shared, **{k: v[c] for k, v in per_core.items()}} for c in range(N_CORES)
    ]
    res = bass_utils.run_bass_kernel_spmd(
        nc, in_maps, core_ids=list(range(N_CORES)), trace=trace
    )
    out = np.concatenate([np.ascontiguousarray(r["yT"].T) for r in res.results], axis=0)
    return out.astype(np.float32), res


def kernel(**inputs) -> np.ndarray:
    return run_hw(inputs, trace=False)[0]

